# revision 38
# baseline (speedup 1.0000x reference)
"""Trainium2 Bass kernel for nn_PredictionLatticeCore.

Semantics (from the reference):
    h = x[:, spine, :] * 0.5
    g = sigmoid(...)                 # cancels: g*h + (1-g)*h == h (to f32 rounding)
    update = LayerNorm(h) * ln_g + ln_b
    out = x with rows at spine replaced by update

Strategy: data-parallel over the flattened (B*S) row dim — 8 contiguous
2048-row chunks, one per core.  Each core DMA-copies its 16.8MB chunk
input->output (the memory-bound bulk) and computes the <=10 LayerNorm
update rows that fall in its chunk (host-gathered, 96KB in) on-chip.
Host overlays the updated rows while unsharding.
"""

import numpy as np

B = 2
S = 8192
D = 2048
N_CORES = 8
ROWS = B * S             # 16384
CH = ROWS // N_CORES     # 2048 rows per core
KMAX = 10                # max spine rows in one 2048-row chunk (chunks never
                         # straddle a batch, so <= 10 spine rows each)
KDECL = 20               # declared DRAM rows for pars/urows; kept at 20 on
                         # purpose: shrinking these tensors shifts ys' DRAM
                         # base and reproducibly costs ~4.7us (HBM/engine
                         # striping), so keep the layout and just transfer less
LN_EPS = 1e-5

_cache = {}
_PAD = np.zeros(8192, dtype=np.float32)

TRACE = False
LAST_RESULT = None


def _build_program():
    import concourse.bacc as bacc
    import concourse.tile as tile
    from concourse import mybir

    f32 = mybir.dt.float32
    # Bacc (not plain Bass): its finalize() runs generate_event_semaphores,
    # which splits multi-sem waits into 1-wait chains (TRN2 HW limit).
    nc = bacc.Bacc(None, target_bir_lowering=False)
    xs = nc.declare_dram_parameter("xs", [CH * D], f32, isOutput=False)
    pars = nc.declare_dram_parameter("pars", [KDECL, 3 * D], f32, isOutput=False)
    # 32KB pad aligns ys to a 64KB multiple from xs's base (offset 264x64KB
    # instead of 263.5): DMA descriptors stop straddling HBM channel stripes.
    nc.declare_dram_parameter("padq", [8192], f32, isOutput=False)
    ys = nc.declare_dram_parameter("ys", [CH * D], f32, isOutput=True)
    urows = nc.declare_dram_parameter("urows", [KDECL, D], f32, isOutput=True)

    with tile.TileContext(nc) as tc:
        with tc.tile_pool(name="p", bufs=1) as pool:
            # Params DMA first: HWDGE queues are FIFO, so these must be
            # enqueued ahead of the bulk-copy descriptors or the LayerNorm
            # stalls until the whole 16.8MB copy drains (observed +21us).
            # Rows go on Q_I (10x8KB descs), gamma/beta once on Q_X (1x16KB)
            # and are partition-broadcast on-chip -- 96KB of head traffic
            # instead of 240KB of replicated rows.  (GpSimd SWDGE for these
            # measured +48us -- SWDGE service is not round-robin with HWDGE.)
            p = pool.tile([KMAX, D], f32)
            nc.sync.dma_start(out=p[:], in_=pars[0:KMAX, 0:D])
            gb = pool.tile([1, 2 * D], f32)
            nc.scalar.dma_start(out=gb[:], in_=pars[0:1, D:3 * D])

            # Bulk copy split across both HWDGE trigger engines (SP + Act):
            # two queues halve descriptor-generation ramp; the 16 DMA
            # engines service both queues.
            half = (CH * D) // 2
            nc.sync.dma_start(out=ys[0:half], in_=xs[0:half])
            nc.scalar.dma_start(out=ys[half:CH * D], in_=xs[half:CH * D])

            # Broadcast gamma/beta to all KMAX partitions on the otherwise
            # idle GpSimd engine; also keeps later DVE consumers at a single
            # sync wait (same-engine producer + one cross-engine event).
            g2 = pool.tile([KMAX, D], f32)
            nc.gpsimd.partition_broadcast(g2[:], gb[0:1, 0:D], channels=KMAX)
            bt2 = pool.tile([KMAX, D], f32)
            nc.gpsimd.partition_broadcast(bt2[:], gb[0:1, D:2 * D], channels=KMAX)

            h = pool.tile([KMAX, D], f32)
            nc.scalar.mul(h[:], p[:], 0.5)
            ssum = pool.tile([KMAX, 1], f32)
            nc.vector.reduce_sum(ssum[:], h[:], axis=mybir.AxisListType.X)
            mean = pool.tile([KMAX, 1], f32)
            nc.vector.tensor_scalar_mul(mean[:], ssum[:], 1.0 / D)
            hc = pool.tile([KMAX, D], f32)
            nc.vector.tensor_scalar_sub(hc[:], h[:], mean[:])
            sq = pool.tile([KMAX, D], f32)
            ssq = pool.tile([KMAX, 1], f32)
            nc.scalar.activation(
                sq[:], hc[:], mybir.ActivationFunctionType.Square,
                accum_out=ssq[:],
            )
            var = pool.tile([KMAX, 1], f32)
            nc.vector.tensor_scalar_mul(var[:], ssq[:], 1.0 / D)
            vare = pool.tile([KMAX, 1], f32)
            nc.vector.tensor_scalar_add(vare[:], var[:], LN_EPS)
            std = pool.tile([KMAX, 1], f32)
            nc.scalar.activation(
                std[:], vare[:], mybir.ActivationFunctionType.Sqrt,
            )
            rstd = pool.tile([KMAX, 1], f32)
            nc.vector.reciprocal(rstd[:], std[:])
            normed = pool.tile([KMAX, D], f32)
            nc.vector.tensor_scalar_mul(normed[:], hc[:], rstd[:])
            u = pool.tile([KMAX, D], f32)
            nc.vector.tensor_mul(u[:], normed[:], g2[:])
            u2 = pool.tile([KMAX, D], f32)
            nc.vector.tensor_add(u2[:], u[:], bt2[:])
            nc.sync.dma_start(out=urows[0:KMAX, :], in_=u2[:])
    nc.finalize()
    return nc


def _ensure_ntff_hook():
    """bass_utils' axon trace path does `from antenv.axon_hooks import ...`;
    some images ship antenv without that submodule, turning trace=True (or a
    BASS_TRACE env set by a caller) into a crash.  Shim the module and, when
    possible, install the real ctypes hook so tracing actually works."""
    import sys
    import types

    try:
        import antenv.axon_hooks  # noqa: F401
        return
    except ImportError:
        pass
    try:
        import antenv
    except ImportError:
        antenv = types.ModuleType("antenv")
        sys.modules["antenv"] = antenv
    mod = types.ModuleType("antenv.axon_hooks")
    mod._hook = None
    mod.set_axon_ntff_profile_hook = lambda h: setattr(mod, "_hook", h)
    mod.get_axon_ntff_profile_hook = lambda: mod._hook
    sys.modules["antenv.axon_hooks"] = mod
    antenv.axon_hooks = mod
    try:
        from trn_agent_boot.trn_boot import _ntff_profile_via_ctypes

        hook = _ntff_profile_via_ctypes("/opt/axon/libaxon_pjrt.so")
        if hook is not None:
            mod._hook = hook
    except Exception:
        pass


def kernel(x, gate_w=None, gate_b=None, ln_g=None, ln_b=None, spine=None):
    global LAST_RESULT
    _ensure_ntff_hook()
    from concourse.bass_utils import run_bass_kernel_spmd

    x = np.asarray(x, dtype=np.float32)
    ln_g = np.asarray(ln_g, dtype=np.float32)
    ln_b = np.asarray(ln_b, dtype=np.float32)
    spine = np.asarray(spine).astype(np.int64)

    if "nc" not in _cache:
        _cache["nc"] = _build_program()
    nc = _cache["nc"]

    xf = np.ascontiguousarray(x).reshape(ROWS, D)
    flat = (np.arange(B, dtype=np.int64)[:, None] * S + spine[None, :]).reshape(-1)

    # Each core LayerNorms only the spine rows inside its own chunk (padded
    # to KMAX); halves param traffic vs. replicating all rows everywhere.
    core_rows = [flat[(flat >= c * CH) & (flat < (c + 1) * CH)]
                 for c in range(N_CORES)]
    in_maps = []
    for c in range(N_CORES):
        pars_np = np.zeros((KDECL, 3 * D), dtype=np.float32)
        r = core_rows[c]
        pars_np[:len(r), 0:D] = xf[r]
        pars_np[0, D:2 * D] = ln_g
        pars_np[0, 2 * D:3 * D] = ln_b
        in_maps.append({
            "xs": xf[c * CH:(c + 1) * CH].reshape(-1),
            "pars": pars_np,
            "padq": _PAD,
        })
    out = run_bass_kernel_spmd(nc, in_maps, list(range(N_CORES)), trace=TRACE)
    if TRACE:
        LAST_RESULT = out
    res = out.results

    y = np.empty((ROWS, D), dtype=np.float32)
    for c in range(N_CORES):
        y[c * CH:(c + 1) * CH] = res[c]["ys"].reshape(CH, D)
        r = core_rows[c]
        if len(r):
            y[r] = res[c]["urows"][:len(r)]
    return y.reshape(B, S, D)


# revision 41
# speedup vs baseline: 1.1400x; 1.1400x over previous
"""Trainium2 Bass kernel for nn_PredictionLatticeCore.

Semantics (from the reference):
    h = x[:, spine, :] * 0.5
    g = sigmoid(...)                 # cancels: g*h + (1-g)*h == h (to f32 rounding)
    update = LayerNorm(h) * ln_g + ln_b
    out = x with rows at spine replaced by update

Strategy: data-parallel over the flattened (B*S) row dim — 8 contiguous
2048-row chunks, one per core.  Each core DMA-copies its 16.8MB chunk
input->output (the memory-bound bulk) and computes the <=10 LayerNorm
update rows that fall in its chunk (host-gathered, 96KB in) on-chip.
Host overlays the updated rows while unsharding.
"""

import numpy as np

B = 2
S = 8192
D = 2048
N_CORES = 8
ROWS = B * S             # 16384
CH = ROWS // N_CORES     # 2048 rows per core
KMAX = 10                # max spine rows in one 2048-row chunk (chunks never
                         # straddle a batch, so <= 10 spine rows each)
KDECL = 20               # declared DRAM rows for pars/urows; kept at 20 on
                         # purpose: shrinking these tensors shifts ys' DRAM
                         # base and reproducibly costs ~4.7us (HBM/engine
                         # striping), so keep the layout and just transfer less
LN_EPS = 1e-5

_cache = {}

TRACE = False
LAST_RESULT = None


def _build_program():
    import concourse.bacc as bacc
    import concourse.tile as tile
    from concourse import mybir

    f32 = mybir.dt.float32
    # Bacc (not plain Bass): its finalize() runs generate_event_semaphores,
    # which splits multi-sem waits into 1-wait chains (TRN2 HW limit).
    nc = bacc.Bacc(None, target_bir_lowering=False)
    xs = nc.declare_dram_parameter("xs", [CH * D], f32, isOutput=False)
    pars = nc.declare_dram_parameter("pars", [KDECL, 3 * D], f32, isOutput=False)
    ys = nc.declare_dram_parameter("ys", [CH * D], f32, isOutput=True)
    urows = nc.declare_dram_parameter("urows", [KDECL, D], f32, isOutput=True)

    with tile.TileContext(nc) as tc:
        with tc.tile_pool(name="p", bufs=1) as pool:
            # Params DMA first: HWDGE queues are FIFO, so these must be
            # enqueued ahead of the bulk-copy descriptors or the LayerNorm
            # stalls until the whole 16.8MB copy drains (observed +21us).
            # Rows go on Q_I (10x8KB descs), gamma/beta once on Q_X (1x16KB)
            # and are partition-broadcast on-chip -- 96KB of head traffic
            # instead of 240KB of replicated rows.  (GpSimd SWDGE for these
            # measured +48us -- SWDGE service is not round-robin with HWDGE.)
            p = pool.tile([KMAX, D], f32)
            nc.sync.dma_start(out=p[:], in_=pars[0:KMAX, 0:D])
            gb = pool.tile([1, 2 * D], f32)
            nc.scalar.dma_start(out=gb[:], in_=pars[0:1, D:3 * D])

            # Bulk copy split across both HWDGE trigger engines (SP + Act):
            # two queues halve descriptor-generation ramp; the 16 DMA
            # engines service both queues.
            half = (CH * D) // 2
            nc.sync.dma_start(out=ys[0:half], in_=xs[0:half])
            nc.scalar.dma_start(out=ys[half:CH * D], in_=xs[half:CH * D])

            # Broadcast gamma/beta to all KMAX partitions on the otherwise
            # idle GpSimd engine; also keeps later DVE consumers at a single
            # sync wait (same-engine producer + one cross-engine event).
            g2 = pool.tile([KMAX, D], f32)
            nc.gpsimd.partition_broadcast(g2[:], gb[0:1, 0:D], channels=KMAX)
            bt2 = pool.tile([KMAX, D], f32)
            nc.gpsimd.partition_broadcast(bt2[:], gb[0:1, D:2 * D], channels=KMAX)

            h = pool.tile([KMAX, D], f32)
            nc.scalar.mul(h[:], p[:], 0.5)
            ssum = pool.tile([KMAX, 1], f32)
            nc.vector.reduce_sum(ssum[:], h[:], axis=mybir.AxisListType.X)
            mean = pool.tile([KMAX, 1], f32)
            nc.vector.tensor_scalar_mul(mean[:], ssum[:], 1.0 / D)
            hc = pool.tile([KMAX, D], f32)
            nc.vector.tensor_scalar_sub(hc[:], h[:], mean[:])
            sq = pool.tile([KMAX, D], f32)
            ssq = pool.tile([KMAX, 1], f32)
            nc.scalar.activation(
                sq[:], hc[:], mybir.ActivationFunctionType.Square,
                accum_out=ssq[:],
            )
            var = pool.tile([KMAX, 1], f32)
            nc.vector.tensor_scalar_mul(var[:], ssq[:], 1.0 / D)
            vare = pool.tile([KMAX, 1], f32)
            nc.vector.tensor_scalar_add(vare[:], var[:], LN_EPS)
            std = pool.tile([KMAX, 1], f32)
            nc.scalar.activation(
                std[:], vare[:], mybir.ActivationFunctionType.Sqrt,
            )
            rstd = pool.tile([KMAX, 1], f32)
            nc.vector.reciprocal(rstd[:], std[:])
            normed = pool.tile([KMAX, D], f32)
            nc.vector.tensor_scalar_mul(normed[:], hc[:], rstd[:])
            u = pool.tile([KMAX, D], f32)
            nc.vector.tensor_mul(u[:], normed[:], g2[:])
            u2 = pool.tile([KMAX, D], f32)
            nc.vector.tensor_add(u2[:], u[:], bt2[:])
            nc.sync.dma_start(out=urows[0:KMAX, :], in_=u2[:])
    nc.finalize()
    return nc


def _ensure_ntff_hook():
    """bass_utils' axon trace path does `from antenv.axon_hooks import ...`;
    some images ship antenv without that submodule, turning trace=True (or a
    BASS_TRACE env set by a caller) into a crash.  Shim the module and, when
    possible, install the real ctypes hook so tracing actually works."""
    import sys
    import types

    try:
        import antenv.axon_hooks  # noqa: F401
        return
    except ImportError:
        pass
    try:
        import antenv
    except ImportError:
        antenv = types.ModuleType("antenv")
        sys.modules["antenv"] = antenv
    mod = types.ModuleType("antenv.axon_hooks")
    mod._hook = None
    mod.set_axon_ntff_profile_hook = lambda h: setattr(mod, "_hook", h)
    mod.get_axon_ntff_profile_hook = lambda: mod._hook
    sys.modules["antenv.axon_hooks"] = mod
    antenv.axon_hooks = mod
    try:
        from trn_agent_boot.trn_boot import _ntff_profile_via_ctypes

        hook = _ntff_profile_via_ctypes("/opt/axon/libaxon_pjrt.so")
        if hook is not None:
            mod._hook = hook
    except Exception:
        pass


def kernel(x, gate_w=None, gate_b=None, ln_g=None, ln_b=None, spine=None):
    global LAST_RESULT
    _ensure_ntff_hook()
    from concourse.bass_utils import run_bass_kernel_spmd

    x = np.asarray(x, dtype=np.float32)
    ln_g = np.asarray(ln_g, dtype=np.float32)
    ln_b = np.asarray(ln_b, dtype=np.float32)
    spine = np.asarray(spine).astype(np.int64)

    if "nc" not in _cache:
        _cache["nc"] = _build_program()
    nc = _cache["nc"]

    xf = np.ascontiguousarray(x).reshape(ROWS, D)
    flat = (np.arange(B, dtype=np.int64)[:, None] * S + spine[None, :]).reshape(-1)

    # Each core LayerNorms only the spine rows inside its own chunk (padded
    # to KMAX); halves param traffic vs. replicating all rows everywhere.
    core_rows = [flat[(flat >= c * CH) & (flat < (c + 1) * CH)]
                 for c in range(N_CORES)]
    in_maps = []
    for c in range(N_CORES):
        pars_np = np.zeros((KDECL, 3 * D), dtype=np.float32)
        r = core_rows[c]
        pars_np[:len(r), 0:D] = xf[r]
        pars_np[0, D:2 * D] = ln_g
        pars_np[0, 2 * D:3 * D] = ln_b
        in_maps.append({
            "xs": xf[c * CH:(c + 1) * CH].reshape(-1),
            "pars": pars_np,
        })
    out = run_bass_kernel_spmd(nc, in_maps, list(range(N_CORES)), trace=TRACE)
    if TRACE:
        LAST_RESULT = out
    res = out.results

    y = np.empty((ROWS, D), dtype=np.float32)
    for c in range(N_CORES):
        y[c * CH:(c + 1) * CH] = res[c]["ys"].reshape(CH, D)
        r = core_rows[c]
        if len(r):
            y[r] = res[c]["urows"][:len(r)]
    return y.reshape(B, S, D)


# revision 42
# speedup vs baseline: 1.1423x; 1.0020x over previous
"""Trainium2 Bass kernel for nn_PredictionLatticeCore.

Semantics (from the reference):
    h = x[:, spine, :] * 0.5
    g = sigmoid(...)                 # cancels: g*h + (1-g)*h == h (to f32 rounding)
    update = LayerNorm(h) * ln_g + ln_b
    out = x with rows at spine replaced by update

Strategy: data-parallel over the flattened (B*S) row dim — 8 contiguous
2048-row chunks, one per core.  Each core DMA-copies its 16.8MB chunk
input->output (the memory-bound bulk) and computes the <=10 LayerNorm
update rows that fall in its chunk (host-gathered, 96KB in) on-chip.
Host overlays the updated rows while unsharding.
"""

import numpy as np

B = 2
S = 8192
D = 2048
N_CORES = 8
ROWS = B * S             # 16384
CH = ROWS // N_CORES     # 2048 rows per core
KMAX = 10                # max spine rows in one 2048-row chunk (chunks never
                         # straddle a batch, so <= 10 spine rows each)
KDECL = 20               # declared DRAM rows for pars/urows; kept at 20 on
                         # purpose: shrinking these tensors shifts ys' DRAM
                         # base and reproducibly costs ~4.7us (HBM/engine
                         # striping), so keep the layout and just transfer less
LN_EPS = 1e-5

_cache = {}

TRACE = False
LAST_RESULT = None


def _build_program():
    import concourse.bacc as bacc
    import concourse.tile as tile
    from concourse import mybir

    f32 = mybir.dt.float32
    # Bacc (not plain Bass): its finalize() runs generate_event_semaphores,
    # which splits multi-sem waits into 1-wait chains (TRN2 HW limit).
    nc = bacc.Bacc(None, target_bir_lowering=False)
    xs = nc.declare_dram_parameter("xs", [CH * D], f32, isOutput=False)
    pars = nc.declare_dram_parameter("pars", [KDECL, 3 * D], f32, isOutput=False)
    ys = nc.declare_dram_parameter("ys", [CH * D], f32, isOutput=True)
    urows = nc.declare_dram_parameter("urows", [KDECL, D], f32, isOutput=True)

    with tile.TileContext(nc) as tc:
        with tc.tile_pool(name="p", bufs=1) as pool:
            # Params DMA first: HWDGE queues are FIFO, so these must be
            # enqueued ahead of the bulk-copy descriptors or the LayerNorm
            # stalls until the whole 16.8MB copy drains (observed +21us).
            # Rows go on Q_I (10x8KB descs), gamma/beta once on Q_X (1x16KB)
            # and are partition-broadcast on-chip -- 96KB of head traffic
            # instead of 240KB of replicated rows.  (GpSimd SWDGE for these
            # measured +48us -- SWDGE service is not round-robin with HWDGE.)
            p = pool.tile([KMAX, D], f32)
            nc.sync.dma_start(out=p[:], in_=pars[0:KMAX, 0:D])
            gb = pool.tile([1, 2 * D], f32)
            nc.scalar.dma_start(out=gb[:], in_=pars[0:1, D:3 * D])

            # Bulk copy split across both HWDGE trigger engines (SP + Act):
            # two queues halve descriptor-generation ramp; the 16 DMA
            # engines service both queues.
            half = (CH * D) // 2
            nc.sync.dma_start(out=ys[0:half], in_=xs[0:half])
            nc.scalar.dma_start(out=ys[half:CH * D], in_=xs[half:CH * D])

            # Broadcast gamma/beta to all KMAX partitions on the otherwise
            # idle GpSimd engine; also keeps later DVE consumers at a single
            # sync wait (same-engine producer + one cross-engine event).
            g2 = pool.tile([KMAX, D], f32)
            nc.gpsimd.partition_broadcast(g2[:], gb[0:1, 0:D], channels=KMAX)
            bt2 = pool.tile([KMAX, D], f32)
            nc.gpsimd.partition_broadcast(bt2[:], gb[0:1, D:2 * D], channels=KMAX)

            h = pool.tile([KMAX, D], f32)
            nc.scalar.mul(h[:], p[:], 0.5)
            ssum = pool.tile([KMAX, 1], f32)
            nc.vector.reduce_sum(ssum[:], h[:], axis=mybir.AxisListType.X)
            mean = pool.tile([KMAX, 1], f32)
            nc.vector.tensor_scalar_mul(mean[:], ssum[:], 1.0 / D)
            hc = pool.tile([KMAX, D], f32)
            nc.vector.tensor_scalar_sub(hc[:], h[:], mean[:])
            sq = pool.tile([KMAX, D], f32)
            ssq = pool.tile([KMAX, 1], f32)
            nc.scalar.activation(
                sq[:], hc[:], mybir.ActivationFunctionType.Square,
                accum_out=ssq[:],
            )
            var = pool.tile([KMAX, 1], f32)
            nc.vector.tensor_scalar_mul(var[:], ssq[:], 1.0 / D)
            vare = pool.tile([KMAX, 1], f32)
            nc.vector.tensor_scalar_add(vare[:], var[:], LN_EPS)
            std = pool.tile([KMAX, 1], f32)
            nc.scalar.activation(
                std[:], vare[:], mybir.ActivationFunctionType.Sqrt,
            )
            rstd = pool.tile([KMAX, 1], f32)
            nc.vector.reciprocal(rstd[:], std[:])
            normed = pool.tile([KMAX, D], f32)
            nc.vector.tensor_scalar_mul(normed[:], hc[:], rstd[:])
            u = pool.tile([KMAX, D], f32)
            nc.vector.tensor_mul(u[:], normed[:], g2[:])
            u2 = pool.tile([KMAX, D], f32)
            nc.vector.tensor_add(u2[:], u[:], bt2[:])
            # urows goes on Q_X: descriptor GENERATION (~390ns/desc/queue) paces
            # the whole transfer, and Q_I already carries rows+bulk (138 descs).
            # Putting urows here balances the queues 138/139 instead of 148/129;
            # the 10 tail descs on the long queue were costing ~3-4us.
            nc.scalar.dma_start(out=urows[0:KMAX, :], in_=u2[:])
    nc.finalize()
    return nc


def _ensure_ntff_hook():
    """bass_utils' axon trace path does `from antenv.axon_hooks import ...`;
    some images ship antenv without that submodule, turning trace=True (or a
    BASS_TRACE env set by a caller) into a crash.  Shim the module and, when
    possible, install the real ctypes hook so tracing actually works."""
    import sys
    import types

    try:
        import antenv.axon_hooks  # noqa: F401
        return
    except ImportError:
        pass
    try:
        import antenv
    except ImportError:
        antenv = types.ModuleType("antenv")
        sys.modules["antenv"] = antenv
    mod = types.ModuleType("antenv.axon_hooks")
    mod._hook = None
    mod.set_axon_ntff_profile_hook = lambda h: setattr(mod, "_hook", h)
    mod.get_axon_ntff_profile_hook = lambda: mod._hook
    sys.modules["antenv.axon_hooks"] = mod
    antenv.axon_hooks = mod
    try:
        from trn_agent_boot.trn_boot import _ntff_profile_via_ctypes

        hook = _ntff_profile_via_ctypes("/opt/axon/libaxon_pjrt.so")
        if hook is not None:
            mod._hook = hook
    except Exception:
        pass


def kernel(x, gate_w=None, gate_b=None, ln_g=None, ln_b=None, spine=None):
    global LAST_RESULT
    _ensure_ntff_hook()
    from concourse.bass_utils import run_bass_kernel_spmd

    x = np.asarray(x, dtype=np.float32)
    ln_g = np.asarray(ln_g, dtype=np.float32)
    ln_b = np.asarray(ln_b, dtype=np.float32)
    spine = np.asarray(spine).astype(np.int64)

    if "nc" not in _cache:
        _cache["nc"] = _build_program()
    nc = _cache["nc"]

    xf = np.ascontiguousarray(x).reshape(ROWS, D)
    flat = (np.arange(B, dtype=np.int64)[:, None] * S + spine[None, :]).reshape(-1)

    # Each core LayerNorms only the spine rows inside its own chunk (padded
    # to KMAX); halves param traffic vs. replicating all rows everywhere.
    core_rows = [flat[(flat >= c * CH) & (flat < (c + 1) * CH)]
                 for c in range(N_CORES)]
    in_maps = []
    for c in range(N_CORES):
        pars_np = np.zeros((KDECL, 3 * D), dtype=np.float32)
        r = core_rows[c]
        pars_np[:len(r), 0:D] = xf[r]
        pars_np[0, D:2 * D] = ln_g
        pars_np[0, 2 * D:3 * D] = ln_b
        in_maps.append({
            "xs": xf[c * CH:(c + 1) * CH].reshape(-1),
            "pars": pars_np,
        })
    out = run_bass_kernel_spmd(nc, in_maps, list(range(N_CORES)), trace=TRACE)
    if TRACE:
        LAST_RESULT = out
    res = out.results

    y = np.empty((ROWS, D), dtype=np.float32)
    for c in range(N_CORES):
        y[c * CH:(c + 1) * CH] = res[c]["ys"].reshape(CH, D)
        r = core_rows[c]
        if len(r):
            y[r] = res[c]["urows"][:len(r)]
    return y.reshape(B, S, D)
